# revision 13
# baseline (speedup 1.0000x reference)
"""Self-contained TP-over-heads DeepseekAttention kernel for 8 TRN2 cores.

v2: SBUF-resident Q/K/V (no DRAM spill), weight-streaming passes (Q, K, V)
over x, bf16 probabilities/V/Wo/partials, Pool-engine drains, chunked bf16
ReduceScatter. Each core handles 4 heads end-to-end; host reassembles the
8 per-core [256, 4096] row slices.
"""

import numpy as np
import ml_dtypes

import concourse.bass as bass
import concourse.mybir as mybir
import concourse.tile as tile
from concourse import bacc
from concourse.bass_utils import run_bass_kernel_spmd

# problem shapes (hardcoded per contract)
S = 2048
H = 4096
NH = 32
D = 128
NC = 8
HPC = NH // NC          # 4 heads per core
DPC = HPC * D           # 512 head-dims per core
KT = H // 128           # 32 contraction tiles over hidden
KTH = KT // 2           # kt half
SCH = 512               # s-chunk for projections
NSC = S // SCH          # 4
ST = S // 128           # 16 s-tiles
QCH = 512               # q-chunk in attention
NQC = S // QCH          # 4
NKT = S // 128          # 16 k-tiles in attention
SPC = S // NC           # 256 rows of output per core

f32 = mybir.dt.float32
f32r = mybir.dt.float32r
bf16 = mybir.dt.bfloat16
bf16_np = ml_dtypes.bfloat16

ROPE_THETA = 10000.0
SCALE = float(1.0 / np.sqrt(D))

_CACHE: dict = {}


def _build(with_collective=True, pdt_bf16=True):
    nc = bacc.Bacc("TRN2", target_bir_lowering=False, debug=False, num_devices=NC)

    # ---- I/O ----
    xt = nc.dram_tensor("xt", [KT, 128, S], bf16, kind="ExternalInput").ap()
    wq = nc.dram_tensor("wq", [KT, 128, DPC], bf16, kind="ExternalInput").ap()
    wk = nc.dram_tensor("wk", [KT, 128, DPC], bf16, kind="ExternalInput").ap()
    wv = nc.dram_tensor("wv", [KT, 128, DPC], bf16, kind="ExternalInput").ap()
    wo = nc.dram_tensor("wo", [HPC, 128, H], bf16, kind="ExternalInput").ap()
    cost = nc.dram_tensor("cost", [128, S], f32, kind="ExternalInput").ap()
    sint = nc.dram_tensor("sint", [128, S], f32, kind="ExternalInput").ap()
    rmat = nc.dram_tensor("rmat", [128, 128], f32r, kind="ExternalInput").ap()
    ones_col = nc.dram_tensor("ones_col", [128, 1], f32r, kind="ExternalInput").ap()
    ones_row = nc.dram_tensor("ones_row", [1, 128], f32r, kind="ExternalInput").ap()
    out_ext = nc.dram_tensor("out", [SPC, H], f32, kind="ExternalOutput").ap()

    PDT = bf16 if pdt_bf16 else f32
    xt_p = xt.rearrange("k p s -> p k s")

    with tile.TileContext(nc) as tc:
        with (
            tc.tile_pool(name="dram", bufs=1, space="DRAM") as dram_pool,
            tc.tile_pool(name="store", bufs=1) as store,
        ):
            partial = dram_pool.tile([S, H], PDT, name="partial")
            # AllToAll landing buffer: [qc][src core][64 rows][2][2048]
            a2a_buf = dram_pool.tile([NQC, NC, 64, 2, H // 2], PDT,
                                     name="a2a_buf")

            q_st = store.tile([128, HPC, S], f32r, tag="q_st")
            k_st = store.tile([128, HPC, S], f32r, tag="k_st")
            v_st = store.tile([128, ST, DPC], bf16, tag="v_st")
            rm_sb = store.tile([128, 128], f32r, tag="rm")
            oc_sb = store.tile([128, 1], f32r, tag="oc")
            or_sb = store.tile([1, 128], f32r, tag="or")
            nc.scalar.dma_start(rm_sb[:], rmat[:])
            nc.scalar.dma_start(oc_sb[:], ones_col[:])
            nc.scalar.dma_start(or_sb[:], ones_row[:])

            # ====== projection passes: Q, K then V (weights streamed) =======
            with (
                tc.tile_pool(name="wp", bufs=3) as wpool,
                tc.tile_pool(name="xp", bufs=3) as xpool,
                tc.tile_pool(name="csp", bufs=2) as cspool,
                tc.tile_pool(name="rawp", bufs=3) as rawpool,
                tc.tile_pool(name="ttp", bufs=2) as ttpool,
                tc.tile_pool(name="psA", bufs=4, space="PSUM") as psA,
                tc.tile_pool(name="psR", bufs=2, space="PSUM") as psR,
            ):
                # weight halves stream through a 3-deep ring; allocations are
                # emitted per pass so ring-reuse WAR deps see all readers.
                def load_w_half(wsrc, hf):
                    wt = wpool.tile([128, KTH, DPC], bf16, tag="w")
                    nc.sync.dma_start(
                        wt[:],
                        wsrc.rearrange("k p n -> p k n")[
                            :, KTH * hf:KTH * (hf + 1), :],
                    )
                    return wt

                # prefetch plan: [Q: wq0, wq1, wk0] [K: wk1, wv0] [V: wv1]
                pending = [(wq, 0), (wq, 1), (wk, 0), (wk, 1),
                           (wv, 0), (wv, 1)]
                loaded = [load_w_half(*pending[i]) for i in range(3)]
                next_load = 3

                # --- Q and K passes: out [128 odim, s] + RoPE -> q_st/k_st
                for pi, dst in ((0, q_st), (1, k_st)):
                    wlo, whi = loaded[2 * pi], loaded[2 * pi + 1]
                    for sc in range(NSC):
                        s0 = sc * SCH
                        cos_sb = cspool.tile([128, SCH], f32, tag="cos")
                        sin_sb = cspool.tile([128, SCH], f32, tag="sin")
                        nc.scalar.dma_start(cos_sb[:], cost[:, s0:s0 + SCH])
                        nc.scalar.dma_start(sin_sb[:], sint[:, s0:s0 + SCH])
                        x_lo = xpool.tile([128, KTH, SCH], bf16, tag="x")
                        nc.gpsimd.dma_start(x_lo[:], xt_p[:, 0:KTH, s0:s0 + SCH])
                        x_hi = xpool.tile([128, KTH, SCH], bf16, tag="x")
                        nc.gpsimd.dma_start(x_hi[:], xt_p[:, KTH:KT, s0:s0 + SCH])

                        pss = [psA.tile([128, SCH], f32, tag="proj", name=f"pj{_h}")
                               for _h in range(HPC)]
                        for wt, x_sb, base in ((wlo, x_lo, 0), (whi, x_hi, KTH)):
                            for kt in range(KTH):
                                for h in range(HPC):
                                    nc.tensor.matmul(
                                        pss[h][:],
                                        wt[:, kt, h * 128:(h + 1) * 128],
                                        x_sb[:, kt, :],
                                        start=(base + kt == 0),
                                        stop=(base + kt == KT - 1),
                                    )
                        for h in range(HPC):
                            raw = rawpool.tile([128, SCH], f32r, tag="raw")
                            nc.scalar.copy(raw[:], pss[h][:])
                            psr = psR.tile([128, SCH], f32, tag="rot")
                            nc.tensor.matmul(psr[:], rm_sb[:], raw[:],
                                             start=True, stop=True)
                            t1 = ttpool.tile([128, SCH], f32, tag="t1")
                            nc.vector.tensor_mul(t1[:], raw[:], cos_sb[:])
                            t2 = ttpool.tile([128, SCH], f32, tag="t2")
                            nc.vector.tensor_mul(t2[:], psr[:], sin_sb[:])
                            nc.vector.tensor_add(
                                dst[:, h, s0:s0 + SCH], t1[:], t2[:])

                    # emit next pass's weight loads now that this pass's
                    # readers exist (ring WAR deps double as prefetch)
                    n_pref = 2 if pi == 0 else 1
                    for _ in range(n_pref):
                        loaded.append(load_w_half(*pending[next_load]))
                        next_load += 1

                # --- V pass: natural [s, d] layout -> v_st (bf16)
                wlo, whi = loaded[4], loaded[5]
                with tc.tile_pool(name="psV", bufs=2, space="PSUM") as psV:
                    for sc in range(NSC):
                        s0 = sc * SCH
                        x_lo = xpool.tile([128, KTH, SCH], bf16, tag="x")
                        nc.gpsimd.dma_start(x_lo[:], xt_p[:, 0:KTH, s0:s0 + SCH])
                        x_hi = xpool.tile([128, KTH, SCH], bf16, tag="x")
                        nc.gpsimd.dma_start(x_hi[:], xt_p[:, KTH:KT, s0:s0 + SCH])
                        for stl in range(SCH // 128):
                            st = sc * (SCH // 128) + stl
                            ps = psV.tile([128, DPC], f32, tag="vp")
                            for wt, x_sb, base in ((wlo, x_lo, 0),
                                                   (whi, x_hi, KTH)):
                                for kt in range(KTH):
                                    nc.tensor.matmul(
                                        ps[:],
                                        x_sb[:, kt, stl * 128:(stl + 1) * 128],
                                        wt[:, kt, :],
                                        start=(base + kt == 0),
                                        stop=(base + kt == KT - 1),
                                    )
                            nc.scalar.copy(v_st[:, st, :], ps[:])

            # ====== attention + fused o_proj, qc-outer ======================
            with (
                tc.tile_pool(name="wo", bufs=1) as wo_pool,
                tc.tile_pool(name="pt", bufs=11) as pt_pool,
                tc.tile_pool(name="tmp", bufs=1) as tmp_pool,
                tc.tile_pool(name="attnmisc", bufs=3) as misc_pool,
                tc.tile_pool(name="otp", bufs=8) as ot_pool,
                tc.tile_pool(name="drain", bufs=4) as drain_pool,
                tc.tile_pool(name="red", bufs=2) as red_pool,
                tc.tile_pool(name="psC", bufs=1, space="PSUM") as psC,
            ):
                wo_sb = wo_pool.tile([128, HPC, H], bf16, tag="wo")
                for g in range(4):
                    nc.scalar.dma_start(
                        wo_sb[:, :, 1024 * g:1024 * (g + 1)],
                        wo.rearrange("h p n -> p h n")[:, :,
                                                       1024 * g:1024 * (g + 1)],
                    )

                def emit_oproj_qt(qc_o, qt_local, ots):
                    """One row-tile of o_proj for chunk qc_o: 8 n-chunks x 4
                    heads; drains on Pool, partial writes on the sync queue."""
                    qt = qc_o * (QCH // 128) + qt_local
                    for nci in range(H // 512):
                        n0 = nci * 512
                        ps = psC.tile([128, 512], f32, tag="opx", bufs=2,
                                      name="psop")
                        for h in range(HPC):
                            nc.tensor.matmul(
                                ps[:],
                                ots[h][:, qt_local * 128:(qt_local + 1) * 128],
                                wo_sb[:, h, n0:n0 + 512],
                                start=(h == 0), stop=(h == HPC - 1),
                            )
                        dr = drain_pool.tile([128, 512], PDT, tag="dr")
                        if nci % 2 == 0:
                            nc.vector.tensor_copy(dr[:], ps[:])
                        else:
                            nc.scalar.copy(dr[:], ps[:])
                        nc.sync.dma_start(
                            partial[qt * 128:(qt + 1) * 128, n0:n0 + 512],
                            dr[:],
                        )

                def emit_a2a(qc_o):
                    # exchange 64-row blocks: core d receives its rows of
                    # chunk qc_o from every core (round-robin row ownership)
                    nc.gpsimd.collective_compute(
                        "AllToAll",
                        mybir.AluOpType.bypass,
                        replica_groups=[list(range(NC))],
                        ins=[partial[qc_o * QCH:(qc_o + 1) * QCH, :].opt()],
                        outs=[a2a_buf[qc_o].opt()],
                    )

                def emit_reduce(qc_o):
                    # sum the 8 received blocks; partition p = 2*r + colhalf
                    acc = red_pool.tile([128, H // 2], f32, tag="racc", bufs=2)
                    b0 = red_pool.tile([128, H // 2], PDT, tag="rblk", bufs=3)
                    nc.gpsimd.dma_start(
                        b0[:],
                        a2a_buf.rearrange("q s r h n -> q s (r h) n")[0 + qc_o, 0],
                    )
                    b1 = red_pool.tile([128, H // 2], PDT, tag="rblk", bufs=3)
                    nc.gpsimd.dma_start(
                        b1[:],
                        a2a_buf.rearrange("q s r h n -> q s (r h) n")[0 + qc_o, 1],
                    )
                    nc.vector.tensor_add(acc[:], b0[:], b1[:])
                    for s in range(2, NC):
                        bs = red_pool.tile([128, H // 2], PDT, tag="rblk",
                                           bufs=3)
                        nc.gpsimd.dma_start(
                            bs[:],
                            a2a_buf.rearrange(
                                "q s r h n -> q s (r h) n")[0 + qc_o, s],
                        )
                        nc.vector.tensor_add(acc[:], acc[:], bs[:])
                    nc.gpsimd.dma_start(
                        out_ext.rearrange(
                            "r (h n) -> (r h) n", h=2)[qc_o * 128:
                                                       (qc_o + 1) * 128, :],
                        acc[:],
                    )

                ot_prev = None
                for qc in range(NQC):
                    q0 = qc * QCH
                    ot_cur = []
                    for h in range(HPC):
                        # scores^T in 2-bank pairs + one exp per pair,
                        # interleaved with attn@V accumulation (lag one pair)
                        ps_o = psC.tile([128, QCH], f32, tag="vmm", bufs=2)
                        pts = []
                        for g in range(NKT // 2):
                            ps_s = psC.tile([128, 2 * QCH], f32, tag="scores",
                                            bufs=2)
                            for j in range(2):
                                kt = 2 * g + j
                                nc.tensor.matmul(
                                    ps_s[:, j * QCH:(j + 1) * QCH],
                                    k_st[:, h, kt * 128:(kt + 1) * 128],
                                    q_st[:, h, q0:q0 + QCH],
                                    start=True, stop=True,
                                )
                            pt = pt_pool.tile([128, 2 * QCH], bf16, tag="pt")
                            nc.scalar.activation(
                                pt[:], ps_s[:],
                                mybir.ActivationFunctionType.Exp, scale=SCALE,
                            )
                            pts.append(pt)
                            if g >= 1:
                                for j in range(2):
                                    kv = 2 * (g - 1) + j
                                    nc.tensor.matmul(
                                        ps_o[:],
                                        v_st[:, kv, h * 128:(h + 1) * 128],
                                        pts[g - 1][:, j * QCH:(j + 1) * QCH],
                                        start=(kv == 0), stop=False,
                                    )
                        for j in range(2):
                            kv = NKT - 2 + j
                            nc.tensor.matmul(
                                ps_o[:],
                                v_st[:, kv, h * 128:(h + 1) * 128],
                                pts[NKT // 2 - 1][:, j * QCH:(j + 1) * QCH],
                                start=False, stop=(kv == NKT - 1),
                            )

                        # denominator: batched tree sum of the 8 P^T pairs
                        tmp = tmp_pool.tile([128, 4, 2 * QCH], f32, tag="tr")
                        for i in range(4):
                            nc.vector.tensor_add(tmp[:, i, :],
                                                 pts[2 * i][:], pts[2 * i + 1][:])
                        nc.vector.tensor_add(tmp[:, 0:2, :],
                                             tmp[:, 0:2, :], tmp[:, 2:4, :])
                        nc.vector.tensor_add(tmp[:, 0, :],
                                             tmp[:, 0, :], tmp[:, 1, :])
                        t_sum = misc_pool.tile([128, QCH], f32r, tag="tsum",
                                               bufs=2)
                        nc.vector.tensor_add(t_sum[:], tmp[:, 0, 0:QCH],
                                             tmp[:, 0, QCH:2 * QCH])

                        # cross-partition sum -> broadcast -> reciprocal
                        ps_sum = psC.tile([1, QCH], f32, tag="opx", bufs=2,
                                          name="pssum")
                        nc.tensor.matmul(ps_sum[:], oc_sb[:], t_sum[:],
                                         start=True, stop=True)
                        sum_sb = misc_pool.tile([1, QCH], f32r, tag="sum_sb")
                        nc.vector.tensor_copy(sum_sb[:], ps_sum[:])
                        ps_bc = psC.tile([128, QCH], f32, tag="opx", bufs=2,
                                         name="psbc")
                        nc.tensor.matmul(ps_bc[:], or_sb[:], sum_sb[:],
                                         start=True, stop=True)
                        recip_sb = misc_pool.tile([128, QCH], f32, tag="recip")
                        nc.vector.reciprocal(recip_sb[:], ps_bc[:])

                        ot_t = ot_pool.tile([128, QCH], bf16, tag="ot")
                        nc.vector.tensor_mul(ot_t[:], ps_o[:], recip_sb[:])
                        ot_cur.append(ot_t)

                        # fill PE stalls with previous chunk's o_proj
                        # row-tiles; fire its AllToAll once all rows landed
                        if ot_prev is not None and h < 2:
                            emit_oproj_qt(qc - 1, 2 * h, ot_prev)
                            emit_oproj_qt(qc - 1, 2 * h + 1, ot_prev)
                            if with_collective and h == 1:
                                emit_a2a(qc - 1)
                        if with_collective and h == 2 and qc >= 2:
                            emit_reduce(qc - 2)

                    ot_prev = ot_cur

                # last chunk's o_proj + AllToAll + remaining reductions
                for qt in range(4):
                    emit_oproj_qt(NQC - 1, qt, ot_prev)
                if with_collective:
                    emit_a2a(NQC - 1)
                    emit_reduce(NQC - 2)
                    emit_reduce(NQC - 1)

            # ====== tail (non-collective / f32 debug paths) =================
            if not with_collective:
                if pdt_bf16:
                    with tc.tile_pool(name="cvt2", bufs=2) as cvt2_pool:
                        for i in range(SPC // 128):
                            ld = cvt2_pool.tile([128, H], bf16, tag="c2_ld")
                            nc.sync.dma_start(
                                ld[:], partial[i * 128:(i + 1) * 128, :])
                            cv = cvt2_pool.tile([128, H], f32, tag="c2_f32")
                            nc.vector.tensor_copy(cv[:], ld[:])
                            nc.gpsimd.dma_start(
                                out_ext[i * 128:(i + 1) * 128, :], cv[:])
                else:
                    nc.gpsimd.dma_start(out_ext[:], partial[:SPC, :])
            elif not pdt_bf16:
                nc.gpsimd.dma_start(out_ext[:], rs_out[:])

    nc.compile()
    return nc


def _host_prep(positions, hidden_states, Wq, Wk, Wv, Wo):
    X = np.asarray(hidden_states, dtype=np.float32).reshape(S, H)
    XT = np.ascontiguousarray(X.T).astype(bf16_np).reshape(KT, 128, S)

    pos = np.asarray(positions).astype(np.float32)
    inv_freq = (1.0 / (ROPE_THETA ** (np.arange(0, D, 2, dtype=np.float32) / D)))
    freqs = pos[:, None] * inv_freq[None, :]
    emb = np.concatenate([freqs, freqs], axis=-1)        # [S, D]
    cosT = np.ascontiguousarray(np.cos(emb).astype(np.float32).T)  # [128, S]
    sinT = np.ascontiguousarray(np.sin(emb).astype(np.float32).T)

    rm = np.zeros((128, 128), np.float32)
    idx = np.arange(64)
    rm[64 + idx, idx] = -1.0   # out[0:64]  = -in[64:128]
    rm[idx, 64 + idx] = 1.0    # out[64:128] = in[0:64]

    Wq = np.asarray(Wq, dtype=np.float32)
    Wk = np.asarray(Wk, dtype=np.float32)
    Wv = np.asarray(Wv, dtype=np.float32)
    Wo = np.asarray(Wo, dtype=np.float32)

    in_maps = []
    for c in range(NC):
        sl = slice(DPC * c, DPC * (c + 1))
        wq_c = np.ascontiguousarray(Wq[sl, :].T).astype(bf16_np).reshape(KT, 128, DPC)
        wk_c = np.ascontiguousarray(Wk[sl, :].T).astype(bf16_np).reshape(KT, 128, DPC)
        wv_c = np.ascontiguousarray(Wv[sl, :].T).astype(bf16_np).reshape(KT, 128, DPC)
        wo_c = np.ascontiguousarray(Wo[:, sl].T).astype(bf16_np).reshape(HPC, 128, H)
        in_maps.append({
            "xt": XT, "wq": wq_c, "wk": wk_c, "wv": wv_c, "wo": wo_c,
            "cost": cosT, "sint": sinT, "rmat": rm,
            "ones_col": np.ones((128, 1), np.float32),
            "ones_row": np.ones((1, 128), np.float32),
        })
    return in_maps


def _assemble(results):
    """Reassemble full [1, S, H] output from per-core row slices."""
    # round-robin ownership: global row = qc*QCH + c*64 + r
    full = np.empty((NQC, NC, QCH // NC, H), np.float32)
    for c in range(NC):
        full[:, c] = results[c]["out"].reshape(NQC, QCH // NC, H)
    return full.reshape(1, S, H)


def kernel(positions, hidden_states, Wq, Wk, Wv, Wo):
    if "nc" not in _CACHE:
        _CACHE["nc"] = _build()
    nc = _CACHE["nc"]
    in_maps = _host_prep(positions, hidden_states, Wq, Wk, Wv, Wo)
    res = run_bass_kernel_spmd(nc, in_maps, list(range(NC)))
    return _assemble(res.results).astype(np.float32)
